# revision 2
# baseline (speedup 1.0000x reference)
"""Trainium2 Bass kernel for nn_FDConv (per-sample frequency-domain-synthesized
3x3 grouped conv).

Strategy (data-parallel over batch, 1 sample per NeuronCore):
  - host: permute dft_weight into dense half-spectrum layout (pure gather) and
    precompute DFT basis matrices as constants.
  - device per core:
      att = sigmoid(logits)                      (0.5 att scale folded in basis)
      GT  = (att-mixed spectrum)^T . [C | S]     (stage-1 iFFT along axis 0, via
                                                  16+16 PE matmuls, contraction
                                                  folded with the kernel-mixing)
      T[which,dx] = A_dx^T . GT_re - B_dx^T . GT_im  (stage-2 irfft + reshape to
                                                  six 128x128 conv weight mats)
      conv: x loaded bf16 with even rows on partitions 0-63, odd rows on 64-127;
            output row-pairs (2u+1, 2u+2) computed as 6 K=128/M=128/N=256
            matmuls (T1_dx on slot u, T2_dx on slot u+1) accumulated in PSUM.
  - outputs DMA'd back fp32.
"""

import math
import numpy as np
import ml_dtypes

import concourse.bass as bass
import concourse.bacc as bacc
import concourse.tile as tile
import concourse.mybir as mybir
from concourse.bass_utils import run_bass_kernel_spmd

F32 = mybir.dt.float32
BF16 = mybir.dt.bfloat16

B, CIN, COUT, KS = 8, 64, 64, 3
H, W = 256, 256
KNUM = 4
D1, D2 = COUT * KS, CIN * KS          # 192, 192
D2R = D2 // 2 + 1                     # 97
NF = D1 * D2R                         # 18624

NPAIR = 128          # output row pairs (2u+1, 2u+2), u = -1..127
SLOT = W + 2         # 258: [pad, 256 cols, pad] per row-slot
XFREE = NPAIR * SLOT # xtile free size


def _host_constants():
    fh = np.fft.fftfreq(D1)
    fw = np.fft.rfftfreq(D2)
    dist = np.sqrt(fh[:, None] ** 2 + fw[None, :] ** 2)
    idx = np.argsort(dist.ravel(), kind='stable')
    FH = (idx // D2R).astype(np.int64)
    FW = (idx % D2R).astype(np.int64)
    perm = FH * D2R + FW
    inv = np.empty(NF, dtype=np.int64)
    inv[perm] = np.arange(NF)

    hh = np.arange(D1)
    ang = 2.0 * np.pi * np.outer(hh, hh) / D1
    # att scale 2/KNUM = 0.5 folded into the stage-1 basis
    Cb = (np.cos(ang) * (0.5 / D1)).astype(np.float32)
    Sb = (np.sin(ang) * (0.5 / D1)).astype(np.float32)
    C2 = np.concatenate([Cb[:96], Cb[96:]], axis=1)           # [96, 384]
    S2 = np.concatenate([Sb[:96], Sb[96:]], axis=1)           # [96, 384]

    w_ = np.arange(D2R)
    n_ = np.arange(D2)
    alpha = np.full(D2R, 2.0); alpha[0] = 1.0; alpha[-1] = 1.0
    beta = np.full(D2R, 2.0); beta[0] = 0.0; beta[-1] = 0.0
    ang2 = 2.0 * np.pi * np.outer(w_, n_) / D2
    A = (alpha[:, None] * np.cos(ang2) / D2).astype(np.float32)   # [97, 192]
    Bm = (beta[:, None] * np.sin(ang2) / D2).astype(np.float32)
    ab = np.concatenate(
        [A[:, dx::3] for dx in range(3)] + [-Bm[:, dx::3] for dx in range(3)],
        axis=1,
    ).astype(ml_dtypes.bfloat16)                               # [97, 384]
    return inv, C2, S2, ab


_INV, _C2, _S2, _AB = _host_constants()

# (which, dx) order used in the conv weight loop
_WSEQ = [(0, 0), (1, 0), (0, 1), (1, 1), (0, 2), (1, 2)]
# valid quadrants (J, r, c0) per `which` (c0 = J - r + 2*which)
_QUADS = {
    0: [(0, 0, 0), (1, 0, 1), (1, 1, 0)],
    1: [(0, 0, 2), (0, 1, 1), (1, 1, 2)],
}
_ZQUAD = {0: (0, 1), 1: (1, 0)}  # zero quadrant (J, r)


def _emit_kernel(tc):
    nc = tc.nc
    from contextlib import ExitStack

    x_in = nc.dram_tensor("x_in", [CIN, H, W], F32, kind="ExternalInput").ap()
    lg_in = nc.dram_tensor("lg_in", [1, KNUM], F32, kind="ExternalInput").ap()
    dre_in = nc.dram_tensor("dre_in", [KNUM * D1, D2R], F32, kind="ExternalInput").ap()
    dim_in = nc.dram_tensor("dim_in", [KNUM * D1, D2R], F32, kind="ExternalInput").ap()
    cb_in = nc.dram_tensor("cb_in", [96, 2 * D1], F32, kind="ExternalInput").ap()
    sb_in = nc.dram_tensor("sb_in", [96, 2 * D1], F32, kind="ExternalInput").ap()
    ab_in = nc.dram_tensor("ab_in", [D2R, 6 * 64], BF16, kind="ExternalInput").ap()
    out = nc.dram_tensor("out", [COUT, H, W], F32, kind="ExternalOutput").ap()

    with ExitStack() as ctx:
        cpool = ctx.enter_context(tc.tile_pool(name="cpool", bufs=1))
        xpool = ctx.enter_context(tc.tile_pool(name="xpool", bufs=1))
        spool = ctx.enter_context(tc.tile_pool(name="spool", bufs=2))

        # ---- x load: even rows -> partitions 0-63, odd rows -> 64-127, bf16
        xtile = xpool.tile([128, XFREE], BF16, name="xtile")
        xv = xtile.rearrange("p (t s) -> p t s", s=SLOT)
        # zero the per-slot pad columns (cols 0 and 257 of each slot)
        nc.vector.memset(xv[:, :, 0:SLOT:SLOT - 1], 0.0)
        NCHUNK = 4
        tchunk = NPAIR // NCHUNK
        for c in range(NCHUNK):
            for par in range(2):
                src = x_in[:, 2 * c * tchunk + par: 2 * (c + 1) * tchunk: 2, :]
                dst = xv[64 * par: 64 * par + 64,
                         c * tchunk: (c + 1) * tchunk, 1: 1 + W]
                nc.gpsimd.dma_start(out=dst, in_=src)

        # ---- small input loads
        cb_sb = cpool.tile([96, 2 * D1], F32, name="cb_sb")
        nc.sync.dma_start(out=cb_sb[:], in_=cb_in)
        sb_sb = cpool.tile([96, 2 * D1], F32, name="sb_sb")
        nc.sync.dma_start(out=sb_sb[:], in_=sb_in)
        ab_sb = cpool.tile([D2R, 6 * 64], BF16, name="ab_sb")
        nc.sync.dma_start(out=ab_sb[:], in_=ab_in)
        l_sb = cpool.tile([1, KNUM], F32, name="l_sb")
        nc.sync.dma_start(out=l_sb[:], in_=lg_in)
        dre_sb = cpool.tile([96, 8 * D2R], BF16, name="dre_sb")
        nc.gpsimd.dma_start(
            out=dre_sb.rearrange("p (c w) -> p c w", w=D2R),
            in_=dre_in.rearrange("(c p) w -> p c w", p=96),
        )
        dim_sb = cpool.tile([96, 8 * D2R], BF16, name="dim_sb")
        nc.gpsimd.dma_start(
            out=dim_sb.rearrange("p (c w) -> p c w", w=D2R),
            in_=dim_in.rearrange("(c p) w -> p c w", p=96),
        )

        # ---- att = sigmoid(logits), broadcast to 96 partitions
        sig_sb = cpool.tile([1, KNUM], F32, name="sig_sb")
        nc.scalar.activation(sig_sb[:], l_sb[:], mybir.ActivationFunctionType.Sigmoid)
        att_sb = cpool.tile([96, KNUM], F32, name="att_sb")
        nc.gpsimd.partition_broadcast(att_sb[:], sig_sb[:])

        # ---- attC/attS = att[k] * basis  (chunk ck = 2k + half)
        attC = cpool.tile([96, 8 * D1], BF16, name="attC")
        attS = cpool.tile([96, 8 * D1], BF16, name="attS")
        for ck in range(8):
            k, half = ck // 2, ck % 2
            nc.vector.tensor_scalar_mul(
                attC[:, ck * D1:(ck + 1) * D1],
                cb_sb[:, half * D1:(half + 1) * D1],
                att_sb[:, k:k + 1],
            )
            nc.vector.tensor_scalar_mul(
                attS[:, ck * D1:(ck + 1) * D1],
                sb_sb[:, half * D1:(half + 1) * D1],
                att_sb[:, k:k + 1],
            )
        dimneg_sb = cpool.tile([96, 8 * D2R], BF16, name="dimneg_sb")
        nc.vector.tensor_scalar_mul(dimneg_sb[:], dim_sb[:], -1.0)

        gtre_sb = cpool.tile([D2R, D1 + 2], BF16, name="gtre_sb")
        gtim_sb = cpool.tile([D2R, D1 + 2], BF16, name="gtim_sb")
        t_sb = [cpool.tile([128, 128], BF16, name=f"t_sb_{i}") for i in range(6)]

        with tc.tile_pool(name="dftps", bufs=1, space="PSUM") as dpool:
            # ---- stage 1: GT = F^T . (att*C) etc, contraction over (k, h)
            gtre_ps = dpool.tile([D2R, D1], F32, name="gtre_ps")
            gtim_ps = dpool.tile([D2R, D1], F32, name="gtim_ps")
            for ck in range(8):
                nc.tensor.matmul(
                    gtre_ps[:], dre_sb[:, ck * D2R:(ck + 1) * D2R],
                    attC[:, ck * D1:(ck + 1) * D1],
                    start=(ck == 0), stop=False)
            for ck in range(8):
                nc.tensor.matmul(
                    gtre_ps[:], dimneg_sb[:, ck * D2R:(ck + 1) * D2R],
                    attS[:, ck * D1:(ck + 1) * D1],
                    start=False, stop=(ck == 7))
            for ck in range(8):
                nc.tensor.matmul(
                    gtim_ps[:], dre_sb[:, ck * D2R:(ck + 1) * D2R],
                    attS[:, ck * D1:(ck + 1) * D1],
                    start=(ck == 0), stop=False)
            for ck in range(8):
                nc.tensor.matmul(
                    gtim_ps[:], dim_sb[:, ck * D2R:(ck + 1) * D2R],
                    attC[:, ck * D1:(ck + 1) * D1],
                    start=False, stop=(ck == 7))
            nc.vector.tensor_copy(gtre_sb[:, 0:D1], gtre_ps[:])
            nc.vector.tensor_copy(gtim_sb[:, 0:D1], gtim_ps[:])

            # ---- stage 2: six conv weight matrices T[(ci,j),(co,r)]
            def gview(g, c0):
                return g[:, c0:c0 + D1].rearrange(
                    "w (co th) -> w co th", th=3)[:, :, 0:1]

            for i, (which, dx) in enumerate(_WSEQ):
                t_ps = dpool.tile([128, 128], F32, name="t_ps", bufs=3)
                zj, zr = _ZQUAD[which]
                nc.vector.memset(t_ps[64 * zj:64 * zj + 64, 64 * zr:64 * zr + 64], 0.0)
                for (J, r, c0) in _QUADS[which]:
                    o = t_ps[64 * J:64 * J + 64, 64 * r:64 * r + 64]
                    nc.tensor.matmul(o, ab_sb[:, dx * 64:(dx + 1) * 64],
                                     gview(gtre_sb, c0), start=True, stop=False)
                    nc.tensor.matmul(o, ab_sb[:, (3 + dx) * 64:(4 + dx) * 64],
                                     gview(gtim_sb, c0), start=False, stop=True)
                nc.vector.tensor_copy(t_sb[i][:], t_ps[:])

        # ---- conv over row pairs
        with tc.tile_pool(name="convps", bufs=8, space="PSUM") as cps:
            GRP = 16  # pairs per staging tile / store group
            staging = {}
            mm_cnt = {}
            mm_tot = {}

            def pair_mms(u):
                return [(w, (0 if wh == 0 else 1) + u)
                        for w, (wh, dx) in enumerate(_WSEQ)
                        if (wh == 0 and u >= 0) or (wh == 1 and u <= 126)]

            def emit_block(us):
                tiles = {}
                for u in us:
                    tiles[u] = cps.tile([128, W], F32, name="pair_ps")
                    mm_cnt[u] = 0
                    mm_tot[u] = len(pair_mms(u))
                for w, (wh, dx) in enumerate(_WSEQ):
                    for u in us:
                        if wh == 0 and u < 0:
                            continue
                        if wh == 1 and u > 126:
                            continue
                        slot = u + (0 if wh == 0 else 1)
                        rhs = xtile[:, slot * SLOT + dx: slot * SLOT + dx + W]
                        nc.tensor.matmul(
                            tiles[u][:], t_sb[2 * dx + wh][:], rhs,
                            start=(mm_cnt[u] == 0),
                            stop=(mm_cnt[u] == mm_tot[u] - 1),
                            skip_group_check=True)
                        mm_cnt[u] += 1
                # copy PSUM -> staging (deinterleave happens in the store AP)
                for u in us:
                    gi, si = (u + 1) // GRP, (u + 1) % GRP
                    if gi not in staging:
                        staging[gi] = spool.tile([128, GRP * W], F32, name="staging")
                    st = staging[gi][:, si * W:(si + 1) * W]
                    if u == -1:
                        nc.scalar.copy(st[64:128, :], tiles[u][64:128, :])
                    elif u == 127:
                        nc.scalar.copy(st[0:64, :], tiles[u][0:64, :])
                    elif u % 2 == 0:
                        nc.vector.tensor_copy(st, tiles[u][:])
                    else:
                        nc.scalar.copy(st, tiles[u][:])
                    if si == GRP - 1 or u == 127:
                        emit_stores(gi)

            def emit_stores(gi):
                stg = staging.pop(gi)
                ua = max(0, gi * GRP - 1)           # first full pair in group
                ub = min(126, gi * GRP + GRP - 2)   # last full pair in group
                sv = stg.rearrange("p (g w) -> p g w", w=W)
                if gi == 0:
                    # row 0 from pair u=-1 (slot 0, upper half)
                    nc.sync.dma_start(out=out[:, 0:1, :], in_=sv[64:128, 0:1, :])
                if ub >= ua:
                    G = ub - ua + 1
                    sa = ua + 1 - gi * GRP
                    nc.sync.dma_start(
                        out=out[:, 2 * ua + 1: 2 * ub + 2: 2, :],
                        in_=sv[0:64, sa:sa + G, :])
                    nc.sync.dma_start(
                        out=out[:, 2 * ua + 2: 2 * ub + 3: 2, :],
                        in_=sv[64:128, sa:sa + G, :])
                if gi == 128 // GRP:
                    # row 255 from pair u=127 (lower half)
                    si = 128 % GRP
                    nc.sync.dma_start(out=out[:, H - 1:H, :],
                                      in_=sv[0:64, si:si + 1, :])

            emit_block([-1])
            BLK = 4
            for b0 in range(0, 128, BLK):
                emit_block(list(range(b0, b0 + BLK)))


_NC_CACHE = None


def _build_nc():
    global _NC_CACHE
    if _NC_CACHE is None:
        nc = bacc.Bacc("TRN2", target_bir_lowering=False, debug=False,
                       num_devices=B)
        with tile.TileContext(nc) as tc:
            _emit_kernel(tc)
        nc.compile()
        _NC_CACHE = nc
    return _NC_CACHE


def _in_maps(x, k_att_logits, dft_weight):
    x = np.ascontiguousarray(np.asarray(x, dtype=np.float32))
    lg = np.asarray(k_att_logits, dtype=np.float32)
    dw = np.asarray(dft_weight, dtype=np.float32)
    # host-side gather: dense half-spectrum layout [k, h, w, c]
    dftP = dw[:, _INV, :].reshape(KNUM, D1, D2R, 2)
    dre = np.ascontiguousarray(dftP[..., 0].reshape(KNUM * D1, D2R))
    dim = np.ascontiguousarray(dftP[..., 1].reshape(KNUM * D1, D2R))
    maps = []
    for b in range(B):
        maps.append({
            "x_in": x[b],
            "lg_in": lg[b:b + 1],
            "dre_in": dre,
            "dim_in": dim,
            "cb_in": _C2,
            "sb_in": _S2,
            "ab_in": _AB,
        })
    return maps


def _execute(x, k_att_logits, dft_weight, trace=False, **trace_kwargs):
    nc = _build_nc()
    res = run_bass_kernel_spmd(
        nc, _in_maps(x, k_att_logits, dft_weight),
        core_ids=list(range(B)), trace=trace, **trace_kwargs)
    out = np.stack([r["out"] for r in res.results], axis=0)
    return out, res


def kernel(x, k_att_logits, dft_weight):
    out, _ = _execute(x, k_att_logits, dft_weight)
    return out.astype(np.float32)


# revision 4
# speedup vs baseline: 1.0797x; 1.0797x over previous
"""Trainium2 Bass kernel for nn_FDConv (per-sample frequency-domain-synthesized
3x3 grouped conv).

Strategy (data-parallel over batch, 1 sample per NeuronCore):
  - host: permute dft_weight into dense half-spectrum layout (pure gather) and
    precompute DFT basis matrices as constants.
  - device per core:
      att = sigmoid(logits)                      (0.5 att scale folded in basis)
      GT  = (att-mixed spectrum)^T . [C | S]     (stage-1 iFFT along axis 0 via
                                                  PE matmuls; kernel-mixing att
                                                  contraction folded in)
      T[which,dx] = A_dx^T . GT_re - B_dx^T . GT_im  (stage-2 irfft + reshape to
                                                  six 128x128 conv weight mats)
      conv: x loaded fp32 via HWDGE with even rows on partitions 0-63 and odd
            rows on 64-127, cast to bf16 on DVE/ACT/POOL; output row pairs
            (2u+1, 2u+2) computed as six K=128/M=128/N=256 matmuls
            (T1_dx on slot u, T2_dx on slot u+1) accumulated in PSUM.
  - outputs copied PSUM->SBUF (DVE/ACT) and DMA'd back fp32.
"""

import numpy as np
import ml_dtypes

import concourse.bass as bass
import concourse.bacc as bacc
import concourse.tile as tile
import concourse.mybir as mybir
from concourse.bass_utils import run_bass_kernel_spmd

F32 = mybir.dt.float32
BF16 = mybir.dt.bfloat16

B, CIN, COUT, KS = 8, 64, 64, 3
H, W = 256, 256
KNUM = 4
D1, D2 = COUT * KS, CIN * KS          # 192, 192
D2R = D2 // 2 + 1                     # 97
NF = D1 * D2R                         # 18624

NPAIR = 128          # output row pairs (2u+1, 2u+2), u = -1..127
SLOT = W + 2         # 258: [pad, 256 cols, pad] per row-slot
CHS = 16             # slots per x chunk
NCH = NPAIR // CHS   # 8 chunks
GRP = 8              # pairs per store group
BLK = 4              # pairs per PE block


def _host_constants():
    fh = np.fft.fftfreq(D1)
    fw = np.fft.rfftfreq(D2)
    dist = np.sqrt(fh[:, None] ** 2 + fw[None, :] ** 2)
    idx = np.argsort(dist.ravel(), kind='stable')
    FH = (idx // D2R).astype(np.int64)
    FW = (idx % D2R).astype(np.int64)
    perm = FH * D2R + FW
    inv = np.empty(NF, dtype=np.int64)
    inv[perm] = np.arange(NF)

    hh = np.arange(D1)
    ang = 2.0 * np.pi * np.outer(hh, hh) / D1
    # att scale 2/KNUM = 0.5 folded into the stage-1 basis
    Cb = (np.cos(ang) * (0.5 / D1)).astype(np.float32)
    Sb = (np.sin(ang) * (0.5 / D1)).astype(np.float32)
    C2 = np.concatenate([Cb[:96], Cb[96:]], axis=1)           # [96, 384]
    S2 = np.concatenate([Sb[:96], Sb[96:]], axis=1)           # [96, 384]

    w_ = np.arange(D2R)
    n_ = np.arange(D2)
    alpha = np.full(D2R, 2.0); alpha[0] = 1.0; alpha[-1] = 1.0
    beta = np.full(D2R, 2.0); beta[0] = 0.0; beta[-1] = 0.0
    ang2 = 2.0 * np.pi * np.outer(w_, n_) / D2
    A = (alpha[:, None] * np.cos(ang2) / D2).astype(np.float32)   # [97, 192]
    Bm = (beta[:, None] * np.sin(ang2) / D2).astype(np.float32)
    ab = np.concatenate(
        [A[:, dx::3] for dx in range(3)] + [-Bm[:, dx::3] for dx in range(3)],
        axis=1,
    ).astype(ml_dtypes.bfloat16)                               # [97, 384]
    return inv, C2, S2, ab


_INV, _C2, _S2, _AB = _host_constants()

# (which, dx) order used in the conv weight loop; t_sb index = 2*dx + which
_WSEQ = [(0, 0), (1, 0), (0, 1), (1, 1), (0, 2), (1, 2)]
# valid quadrants (J, r, c0) per `which` (c0 = J - r + 2*which)
_QUADS = {
    0: [(0, 0, 0), (1, 0, 1), (1, 1, 0)],
    1: [(0, 0, 2), (0, 1, 1), (1, 1, 2)],
}
_ZQUAD = {0: (0, 1), 1: (1, 0)}  # zero quadrant (J, r)


def _emit_kernel(tc):
    nc = tc.nc
    from contextlib import ExitStack

    x_in = nc.dram_tensor("x_in", [CIN, H, W], F32, kind="ExternalInput").ap()
    lg_in = nc.dram_tensor("lg_in", [1, KNUM], F32, kind="ExternalInput").ap()
    dre_in = nc.dram_tensor("dre_in", [KNUM * D1, D2R], F32, kind="ExternalInput").ap()
    dim_in = nc.dram_tensor("dim_in", [KNUM * D1, D2R], F32, kind="ExternalInput").ap()
    cb_in = nc.dram_tensor("cb_in", [96, 2 * D1], F32, kind="ExternalInput").ap()
    sb_in = nc.dram_tensor("sb_in", [96, 2 * D1], F32, kind="ExternalInput").ap()
    ab_in = nc.dram_tensor("ab_in", [D2R, 6 * 64], BF16, kind="ExternalInput").ap()
    out = nc.dram_tensor("out", [COUT, H, W], F32, kind="ExternalOutput").ap()

    with ExitStack() as ctx:
        cpool = ctx.enter_context(tc.tile_pool(name="cpool", bufs=1))
        xfpool = ctx.enter_context(tc.tile_pool(name="xfpool", bufs=3))
        xbpool = ctx.enter_context(tc.tile_pool(name="xbpool", bufs=1))
        spool = ctx.enter_context(tc.tile_pool(name="spool", bufs=3))

        # ---- small input loads (HWDGE, ahead of the x chunks on the queue)
        l_sb = cpool.tile([1, KNUM], F32, name="l_sb")
        nc.sync.dma_start(out=l_sb[:], in_=lg_in)
        dre32 = cpool.tile([96, 8 * D2R], F32, name="dre32")
        nc.sync.dma_start(
            out=dre32.rearrange("p (c w) -> p c w", w=D2R),
            in_=dre_in.rearrange("(c p) w -> p c w", p=96))
        dim32 = cpool.tile([96, 8 * D2R], F32, name="dim32")
        nc.sync.dma_start(
            out=dim32.rearrange("p (c w) -> p c w", w=D2R),
            in_=dim_in.rearrange("(c p) w -> p c w", p=96))
        cb_sb = cpool.tile([96, 2 * D1], F32, name="cb_sb")
        nc.sync.dma_start(out=cb_sb[:], in_=cb_in)
        sb_sb = cpool.tile([96, 2 * D1], F32, name="sb_sb")
        nc.sync.dma_start(out=sb_sb[:], in_=sb_in)
        ab_sb = cpool.tile([D2R, 6 * 64], BF16, name="ab_sb")
        nc.sync.dma_start(out=ab_sb[:], in_=ab_in)

        # ---- att = sigmoid(logits); broadcast via K=1 matmul with ones
        sig_sb = cpool.tile([1, KNUM], F32, name="sig_sb")
        nc.scalar.activation(sig_sb[:], l_sb[:], mybir.ActivationFunctionType.Sigmoid)
        ones_sb = cpool.tile([1, 128], F32, name="ones_sb")
        nc.vector.memset(ones_sb[:], 1.0)

        # ---- x chunk loads (fp32 HWDGE) + casts to interleaved bf16 layout
        xch = []
        for c in range(NCH):
            xb = xbpool.tile([128, (CHS + 1) * SLOT], BF16, name=f"xb_{c}")
            xch.append(xb)
        cast_engines = [nc.vector, nc.gpsimd, nc.scalar]
        xf_tiles = []
        for c in range(NCH):
            xf = xfpool.tile([128, CHS * W], F32, name="xf")
            for par in range(2):
                src = x_in[:, 2 * c * CHS + par: 2 * (c + 1) * CHS: 2, :]
                nc.sync.dma_start(
                    out=xf[64 * par: 64 * par + 64, :].rearrange(
                        "p (t w) -> p t w", w=W),
                    in_=src)
            xf_tiles.append(xf)
        for c in range(NCH):
            xb, xf = xch[c], xf_tiles[c]
            xbv = xb.rearrange("p (t s) -> p t s", s=SLOT)
            eng = cast_engines[c % 3]
            nc.vector.memset(xbv[:, :, 0:SLOT:SLOT - 1], 0.0)
            # main body: slots local 0..CHS-1
            if eng is nc.scalar:
                eng.copy(xbv[:, 0:CHS, 1:1 + W],
                         xf.rearrange("p (t w) -> p t w", w=W))
            else:
                eng.tensor_copy(xbv[:, 0:CHS, 1:1 + W],
                                xf.rearrange("p (t w) -> p t w", w=W))
            # boundary: local slot CHS = first slot of next chunk
            if c + 1 < NCH:
                nxt = xf_tiles[c + 1]
                if eng is nc.scalar:
                    eng.copy(xbv[:, CHS:CHS + 1, 1:1 + W],
                             nxt.rearrange("p (t w) -> p t w", w=W)[:, 0:1, :])
                else:
                    eng.tensor_copy(
                        xbv[:, CHS:CHS + 1, 1:1 + W],
                        nxt.rearrange("p (t w) -> p t w", w=W)[:, 0:1, :])

        def slot_ap(s):
            c, loc = s // CHS, s % CHS
            if c == NCH:           # slot 128 never read (u=127 has no T2)
                c, loc = NCH - 1, CHS
            return xch[c], c, loc

        # ---- bf16 casts of the permuted spectrum
        dre_sb = cpool.tile([96, 8 * D2R], BF16, name="dre_sb")
        nc.vector.tensor_copy(dre_sb[:], dre32[:])
        dim_sb = cpool.tile([96, 8 * D2R], BF16, name="dim_sb")
        nc.vector.tensor_copy(dim_sb[:], dim32[:])
        dimneg_sb = cpool.tile([96, 8 * D2R], BF16, name="dimneg_sb")
        nc.vector.tensor_scalar_mul(dimneg_sb[:], dim32[:], -1.0)

        gtre_sb = cpool.tile([D2R, D1 + 2], BF16, name="gtre_sb")
        gtim_sb = cpool.tile([D2R, D1 + 2], BF16, name="gtim_sb")
        t_sb = [cpool.tile([128, 128], BF16, name=f"t_sb_{i}") for i in range(6)]

        with tc.tile_pool(name="dftps", bufs=1, space="PSUM") as dpool:
            # att broadcast: [128, 4] = ones.T @ sig
            att_ps = dpool.tile([128, KNUM], F32, name="att_ps")
            nc.tensor.matmul(att_ps[:], ones_sb[:], sig_sb[:], start=True, stop=True)
            att_sb = cpool.tile([96, KNUM], F32, name="att_sb")
            nc.vector.tensor_copy(att_sb[:], att_ps[0:96, :])

            # attC on ACT, attS on DVE (chunk ck = 2k + half)
            attC = cpool.tile([96, 8 * D1], BF16, name="attC")
            attS = cpool.tile([96, 8 * D1], BF16, name="attS")
            for ck in range(8):
                k, half = ck // 2, ck % 2
                nc.scalar.mul(
                    attC[:, ck * D1:(ck + 1) * D1],
                    cb_sb[:, half * D1:(half + 1) * D1],
                    att_sb[:, k:k + 1])
                nc.vector.tensor_scalar_mul(
                    attS[:, ck * D1:(ck + 1) * D1],
                    sb_sb[:, half * D1:(half + 1) * D1],
                    att_sb[:, k:k + 1])

            # ---- stage 1: GT = F^T . (att*C) etc, contraction over (k, h)
            gtre_ps = dpool.tile([D2R, D1], F32, name="gtre_ps")
            gtim_ps = dpool.tile([D2R, D1], F32, name="gtim_ps")
            for ck in range(8):
                nc.tensor.matmul(
                    gtre_ps[:], dre_sb[:, ck * D2R:(ck + 1) * D2R],
                    attC[:, ck * D1:(ck + 1) * D1],
                    start=(ck == 0), stop=False)
            for ck in range(8):
                nc.tensor.matmul(
                    gtre_ps[:], dimneg_sb[:, ck * D2R:(ck + 1) * D2R],
                    attS[:, ck * D1:(ck + 1) * D1],
                    start=False, stop=(ck == 7))
            for ck in range(8):
                nc.tensor.matmul(
                    gtim_ps[:], dre_sb[:, ck * D2R:(ck + 1) * D2R],
                    attS[:, ck * D1:(ck + 1) * D1],
                    start=(ck == 0), stop=False)
            for ck in range(8):
                nc.tensor.matmul(
                    gtim_ps[:], dim_sb[:, ck * D2R:(ck + 1) * D2R],
                    attC[:, ck * D1:(ck + 1) * D1],
                    start=False, stop=(ck == 7))
            nc.vector.tensor_copy(gtre_sb[:, 0:D1], gtre_ps[:])
            nc.vector.tensor_copy(gtim_sb[:, 0:D1], gtim_ps[:])

            # ---- stage 2: six conv weight matrices T[(ci,j),(co,r)]
            def gview(g, c0):
                return g[:, c0:c0 + D1].rearrange(
                    "w (co th) -> w co th", th=3)[:, :, 0:1]

            for i, (which, dx) in enumerate(_WSEQ):
                t_ps = dpool.tile([128, 128], F32, name="t_ps", bufs=3)
                zj, zr = _ZQUAD[which]
                nc.vector.memset(t_ps[64 * zj:64 * zj + 64, 64 * zr:64 * zr + 64], 0.0)
                for (J, r, c0) in _QUADS[which]:
                    o = t_ps[64 * J:64 * J + 64, 64 * r:64 * r + 64]
                    nc.tensor.matmul(o, ab_sb[:, dx * 64:(dx + 1) * 64],
                                     gview(gtre_sb, c0), start=True, stop=False)
                    nc.tensor.matmul(o, ab_sb[:, (3 + dx) * 64:(4 + dx) * 64],
                                     gview(gtim_sb, c0), start=False, stop=True)
                nc.vector.tensor_copy(t_sb[2 * dx + which][:], t_ps[:])

        # ---- conv over row pairs
        with tc.tile_pool(name="convps", bufs=8, space="PSUM") as cps:
            staging = {}
            mm_cnt = {}
            mm_tot = {}

            def emit_block(us):
                tiles = {}
                for u in us:
                    tiles[u] = cps.tile([128, W], F32, name="pair_ps")
                    mm_cnt[u] = 0
                    mm_tot[u] = sum(
                        1 for wh, dx in _WSEQ
                        if (wh == 0 and u >= 0) or (wh == 1 and u <= 126))
                for wh, dx in _WSEQ:
                    for u in us:
                        if wh == 0 and u < 0:
                            continue
                        if wh == 1 and u > 126:
                            continue
                        s = u + (0 if wh == 0 else 1)
                        xb, c, loc = slot_ap(s)
                        rhs = xb[:, loc * SLOT + dx: loc * SLOT + dx + W]
                        nc.tensor.matmul(
                            tiles[u][:], t_sb[2 * dx + wh][:], rhs,
                            start=(mm_cnt[u] == 0),
                            stop=(mm_cnt[u] == mm_tot[u] - 1),
                            skip_group_check=True)
                        mm_cnt[u] += 1
                for u in us:
                    gi, si = (u + 1) // GRP, (u + 1) % GRP
                    if gi not in staging:
                        staging[gi] = spool.tile([128, GRP * W], F32, name="staging")
                    st = staging[gi][:, si * W:(si + 1) * W]
                    if u == -1:
                        nc.scalar.copy(st[64:128, :], tiles[u][64:128, :])
                    elif u == 127:
                        nc.scalar.copy(st[0:64, :], tiles[u][0:64, :])
                    elif u % 2 == 0:
                        nc.vector.tensor_copy(st, tiles[u][:])
                    else:
                        nc.scalar.copy(st, tiles[u][:])
                    if si == GRP - 1 or u == 127:
                        emit_stores(gi)

            def emit_stores(gi):
                stg = staging.pop(gi)
                ua = max(0, gi * GRP - 1)           # first full pair in group
                ub = min(126, gi * GRP + GRP - 2)   # last full pair in group
                sv = stg.rearrange("p (g w) -> p g w", w=W)
                if gi == 0:
                    # row 0 from pair u=-1 (slot 0, upper half)
                    nc.sync.dma_start(out=out[:, 0:1, :], in_=sv[64:128, 0:1, :])
                if ub >= ua:
                    G = ub - ua + 1
                    sa = ua + 1 - gi * GRP
                    nc.sync.dma_start(
                        out=out[:, 2 * ua + 1: 2 * ub + 2: 2, :],
                        in_=sv[0:64, sa:sa + G, :])
                    nc.sync.dma_start(
                        out=out[:, 2 * ua + 2: 2 * ub + 3: 2, :],
                        in_=sv[64:128, sa:sa + G, :])
                if gi == 128 // GRP:
                    # row 255 from pair u=127 (lower half)
                    si = 128 % GRP
                    nc.sync.dma_start(out=out[:, H - 1:H, :],
                                      in_=sv[0:64, si:si + 1, :])

            emit_block([-1])
            for b0 in range(0, 128, BLK):
                emit_block(list(range(b0, b0 + BLK)))


_NC_CACHE = None


def _build_nc():
    global _NC_CACHE
    if _NC_CACHE is None:
        nc = bacc.Bacc("TRN2", target_bir_lowering=False, debug=False,
                       num_devices=B)
        with tile.TileContext(nc) as tc:
            _emit_kernel(tc)
        nc.compile()
        _NC_CACHE = nc
    return _NC_CACHE


def _in_maps(x, k_att_logits, dft_weight):
    x = np.ascontiguousarray(np.asarray(x, dtype=np.float32))
    lg = np.asarray(k_att_logits, dtype=np.float32)
    dw = np.asarray(dft_weight, dtype=np.float32)
    # host-side gather: dense half-spectrum layout [k, h, w, c]
    dftP = dw[:, _INV, :].reshape(KNUM, D1, D2R, 2)
    dre = np.ascontiguousarray(dftP[..., 0].reshape(KNUM * D1, D2R))
    dim = np.ascontiguousarray(dftP[..., 1].reshape(KNUM * D1, D2R))
    maps = []
    for b in range(B):
        maps.append({
            "x_in": x[b],
            "lg_in": lg[b:b + 1],
            "dre_in": dre,
            "dim_in": dim,
            "cb_in": _C2,
            "sb_in": _S2,
            "ab_in": _AB,
        })
    return maps


def _execute(x, k_att_logits, dft_weight, trace=False, **trace_kwargs):
    nc = _build_nc()
    res = run_bass_kernel_spmd(
        nc, _in_maps(x, k_att_logits, dft_weight),
        core_ids=list(range(B)), trace=trace, **trace_kwargs)
    out = np.stack([r["out"] for r in res.results], axis=0)
    return out, res


def kernel(x, k_att_logits, dft_weight):
    out, _ = _execute(x, k_att_logits, dft_weight)
    return out.astype(np.float32)


# revision 5
# speedup vs baseline: 1.1159x; 1.0336x over previous
"""Trainium2 Bass kernel for nn_FDConv (per-sample frequency-domain-synthesized
3x3 grouped conv).

Strategy (data-parallel over batch, 1 sample per NeuronCore):
  - host: permute dft_weight into dense half-spectrum layout (pure gather),
    precompute DFT basis matrices as constants, stage x as bf16.
  - device per core:
      att = sigmoid(logits)                      (0.5 att scale folded in basis)
      GT  = (att-mixed spectrum)^T . [C | S]     (stage-1 iFFT along axis 0 via
                                                  PE matmuls; kernel-mixing att
                                                  contraction folded in)
      T[which,dx] = A_dx^T . GT_re - B_dx^T . GT_im  (stage-2 irfft + reshape to
                                                  six 128x128 conv weight mats)
      conv: x bf16 with even rows on partitions 0-63 and odd rows on 64-127;
            output row pairs (2u+1, 2u+2) computed as six K=128/M=128/N=256
            matmuls (T1_dx on slot u, T2_dx on slot u+1) accumulated in PSUM.
  - outputs copied PSUM->SBUF (DVE/ACT) and DMA'd back fp32.
"""

import numpy as np
import ml_dtypes

import concourse.bass as bass
import concourse.bacc as bacc
import concourse.tile as tile
import concourse.mybir as mybir
from concourse.bass_utils import run_bass_kernel_spmd

F32 = mybir.dt.float32
BF16 = mybir.dt.bfloat16

B, CIN, COUT, KS = 8, 64, 64, 3
H, W = 256, 256
KNUM = 4
D1, D2 = COUT * KS, CIN * KS          # 192, 192
D2R = D2 // 2 + 1                     # 97
NF = D1 * D2R                         # 18624

NPAIR = 128          # output row pairs (2u+1, 2u+2), u = -1..127
SLOT = W + 2         # 258: [pad, 256 cols, pad] per row-slot
CHS = 16             # slots per x chunk
NCH = NPAIR // CHS   # 8 chunks
GRP = 16             # pairs per store group
BLK = 4              # pairs per PE block


def _host_constants():
    fh = np.fft.fftfreq(D1)
    fw = np.fft.rfftfreq(D2)
    dist = np.sqrt(fh[:, None] ** 2 + fw[None, :] ** 2)
    idx = np.argsort(dist.ravel(), kind='stable')
    FH = (idx // D2R).astype(np.int64)
    FW = (idx % D2R).astype(np.int64)
    perm = FH * D2R + FW
    inv = np.empty(NF, dtype=np.int64)
    inv[perm] = np.arange(NF)

    hh = np.arange(D1)
    ang = 2.0 * np.pi * np.outer(hh, hh) / D1
    # att scale 2/KNUM = 0.5 folded into the stage-1 basis
    Cb = (np.cos(ang) * (0.5 / D1)).astype(np.float32)
    Sb = (np.sin(ang) * (0.5 / D1)).astype(np.float32)
    C2 = np.concatenate([Cb[:96], Cb[96:]], axis=1)           # [96, 384]
    S2 = np.concatenate([Sb[:96], Sb[96:]], axis=1)           # [96, 384]

    w_ = np.arange(D2R)
    n_ = np.arange(D2)
    alpha = np.full(D2R, 2.0); alpha[0] = 1.0; alpha[-1] = 1.0
    beta = np.full(D2R, 2.0); beta[0] = 0.0; beta[-1] = 0.0
    ang2 = 2.0 * np.pi * np.outer(w_, n_) / D2
    A = (alpha[:, None] * np.cos(ang2) / D2).astype(np.float32)   # [97, 192]
    Bm = (beta[:, None] * np.sin(ang2) / D2).astype(np.float32)
    ab = np.concatenate(
        [A[:, dx::3] for dx in range(3)] + [-Bm[:, dx::3] for dx in range(3)],
        axis=1,
    ).astype(ml_dtypes.bfloat16)                               # [97, 384]
    return inv, C2, S2, ab


_INV, _C2, _S2, _AB = _host_constants()

# (which, dx) order used in the conv weight loop; t_sb index = 2*dx + which
_WSEQ = [(0, 0), (1, 0), (0, 1), (1, 1), (0, 2), (1, 2)]
# valid quadrants (J, r, c0) per `which` (c0 = J - r + 2*which)
_QUADS = {
    0: [(0, 0, 0), (1, 0, 1), (1, 1, 0)],
    1: [(0, 0, 2), (0, 1, 1), (1, 1, 2)],
}
_ZQUAD = {0: (0, 1), 1: (1, 0)}  # zero quadrant (J, r)


def _emit_kernel(tc):
    nc = tc.nc
    from contextlib import ExitStack

    x_in = nc.dram_tensor("x_in", [CIN, H, W], BF16, kind="ExternalInput").ap()
    lg_in = nc.dram_tensor("lg_in", [1, KNUM], F32, kind="ExternalInput").ap()
    dre_in = nc.dram_tensor("dre_in", [KNUM * D1, D2R], F32, kind="ExternalInput").ap()
    dim_in = nc.dram_tensor("dim_in", [KNUM * D1, D2R], F32, kind="ExternalInput").ap()
    cb_in = nc.dram_tensor("cb_in", [96, 2 * D1], F32, kind="ExternalInput").ap()
    sb_in = nc.dram_tensor("sb_in", [96, 2 * D1], F32, kind="ExternalInput").ap()
    ab_in = nc.dram_tensor("ab_in", [D2R, 6 * 64], BF16, kind="ExternalInput").ap()
    out = nc.dram_tensor("out", [COUT, H, W], F32, kind="ExternalOutput").ap()

    with ExitStack() as ctx:
        cpool = ctx.enter_context(tc.tile_pool(name="cpool", bufs=1))
        xbpool = ctx.enter_context(tc.tile_pool(name="xbpool", bufs=4))
        spool = ctx.enter_context(tc.tile_pool(name="spool", bufs=3))

        # ---- small input loads (HWDGE sync queue, private from bulk SWDGE)
        l_sb = cpool.tile([1, KNUM], F32, name="l_sb")
        nc.sync.dma_start(out=l_sb[:], in_=lg_in)
        dre32 = cpool.tile([96, 8 * D2R], F32, name="dre32")
        nc.sync.dma_start(
            out=dre32.rearrange("p (c w) -> p c w", w=D2R),
            in_=dre_in.rearrange("(c p) w -> p c w", p=96))
        dim32 = cpool.tile([96, 8 * D2R], F32, name="dim32")
        nc.sync.dma_start(
            out=dim32.rearrange("p (c w) -> p c w", w=D2R),
            in_=dim_in.rearrange("(c p) w -> p c w", p=96))
        cb_sb = cpool.tile([96, 2 * D1], F32, name="cb_sb")
        nc.sync.dma_start(out=cb_sb[:], in_=cb_in)
        sb_sb = cpool.tile([96, 2 * D1], F32, name="sb_sb")
        nc.sync.dma_start(out=sb_sb[:], in_=sb_in)
        ab_sb = cpool.tile([D2R, 6 * 64], BF16, name="ab_sb")
        nc.sync.dma_start(out=ab_sb[:], in_=ab_in)

        # ---- att = sigmoid(logits); broadcast via K=1 matmul with ones
        sig_sb = cpool.tile([1, KNUM], F32, name="sig_sb")
        nc.scalar.activation(sig_sb[:], l_sb[:], mybir.ActivationFunctionType.Sigmoid)
        ones_sb = cpool.tile([1, 128], F32, name="ones_sb")
        nc.vector.memset(ones_sb[:], 1.0)

        # ---- x chunk loads (bf16 SWDGE): slot t holds rows (2t, 2t+1)
        xch = []
        for c in range(NCH):
            xb = xbpool.tile([128, CHS * SLOT], BF16, name="xb")
            xbv = xb.rearrange("p (t s) -> p t s", s=SLOT)
            nc.vector.memset(xbv[:, :, 0:SLOT:SLOT - 1], 0.0)
            for par in range(2):
                src = x_in[:, 2 * c * CHS + par: 2 * (c + 1) * CHS: 2, :]
                nc.gpsimd.dma_start(
                    out=xbv[64 * par: 64 * par + 64, :, 1:1 + W], in_=src)
            xch.append(xb)

        def slot_rhs(s, dx):
            c, loc = s // CHS, s % CHS
            return xch[c][:, loc * SLOT + dx: loc * SLOT + dx + W]

        # ---- bf16 casts of the permuted spectrum
        dre_sb = cpool.tile([96, 8 * D2R], BF16, name="dre_sb")
        nc.vector.tensor_copy(dre_sb[:], dre32[:])
        dim_sb = cpool.tile([96, 8 * D2R], BF16, name="dim_sb")
        nc.vector.tensor_copy(dim_sb[:], dim32[:])
        dimneg_sb = cpool.tile([96, 8 * D2R], BF16, name="dimneg_sb")
        nc.vector.tensor_scalar_mul(dimneg_sb[:], dim32[:], -1.0)

        gtre_sb = cpool.tile([D2R, D1 + 2], BF16, name="gtre_sb")
        gtim_sb = cpool.tile([D2R, D1 + 2], BF16, name="gtim_sb")
        t_sb = [cpool.tile([128, 128], BF16, name=f"t_sb_{i}") for i in range(6)]

        with tc.tile_pool(name="dftps", bufs=1, space="PSUM") as dpool:
            # att broadcast: [128, 4] = ones.T @ sig
            att_ps = dpool.tile([128, KNUM], F32, name="att_ps")
            nc.tensor.matmul(att_ps[:], ones_sb[:], sig_sb[:], start=True, stop=True)
            att_sb = cpool.tile([96, KNUM], F32, name="att_sb")
            nc.vector.tensor_copy(att_sb[:], att_ps[0:96, :])

            # attC on ACT, attS on DVE (chunk ck = 2k + half)
            attC = cpool.tile([96, 8 * D1], BF16, name="attC")
            attS = cpool.tile([96, 8 * D1], BF16, name="attS")
            for ck in range(8):
                k, half = ck // 2, ck % 2
                nc.scalar.mul(
                    attC[:, ck * D1:(ck + 1) * D1],
                    cb_sb[:, half * D1:(half + 1) * D1],
                    att_sb[:, k:k + 1])
                nc.vector.tensor_scalar_mul(
                    attS[:, ck * D1:(ck + 1) * D1],
                    sb_sb[:, half * D1:(half + 1) * D1],
                    att_sb[:, k:k + 1])

            # ---- stage 1: GT = F^T . (att*C) etc, contraction over (k, h)
            gtre_ps = dpool.tile([D2R, D1], F32, name="gtre_ps")
            gtim_ps = dpool.tile([D2R, D1], F32, name="gtim_ps")
            for ck in range(8):
                nc.tensor.matmul(
                    gtre_ps[:], dre_sb[:, ck * D2R:(ck + 1) * D2R],
                    attC[:, ck * D1:(ck + 1) * D1],
                    start=(ck == 0), stop=False)
            for ck in range(8):
                nc.tensor.matmul(
                    gtre_ps[:], dimneg_sb[:, ck * D2R:(ck + 1) * D2R],
                    attS[:, ck * D1:(ck + 1) * D1],
                    start=False, stop=(ck == 7))
            for ck in range(8):
                nc.tensor.matmul(
                    gtim_ps[:], dre_sb[:, ck * D2R:(ck + 1) * D2R],
                    attS[:, ck * D1:(ck + 1) * D1],
                    start=(ck == 0), stop=False)
            for ck in range(8):
                nc.tensor.matmul(
                    gtim_ps[:], dim_sb[:, ck * D2R:(ck + 1) * D2R],
                    attC[:, ck * D1:(ck + 1) * D1],
                    start=False, stop=(ck == 7))
            nc.vector.tensor_copy(gtre_sb[:, 0:D1], gtre_ps[:])
            nc.vector.tensor_copy(gtim_sb[:, 0:D1], gtim_ps[:])

            # ---- stage 2: six conv weight matrices T[(ci,j),(co,r)]
            def gview(g, c0):
                return g[:, c0:c0 + D1].rearrange(
                    "w (co th) -> w co th", th=3)[:, :, 0:1]

            for i, (which, dx) in enumerate(_WSEQ):
                t_ps = dpool.tile([128, 128], F32, name="t_ps", bufs=3)
                zj, zr = _ZQUAD[which]
                nc.vector.memset(t_ps[64 * zj:64 * zj + 64, 64 * zr:64 * zr + 64], 0.0)
                for (J, r, c0) in _QUADS[which]:
                    o = t_ps[64 * J:64 * J + 64, 64 * r:64 * r + 64]
                    nc.tensor.matmul(o, ab_sb[:, dx * 64:(dx + 1) * 64],
                                     gview(gtre_sb, c0), start=True, stop=False)
                    nc.tensor.matmul(o, ab_sb[:, (3 + dx) * 64:(4 + dx) * 64],
                                     gview(gtim_sb, c0), start=False, stop=True)
                nc.vector.tensor_copy(t_sb[2 * dx + which][:], t_ps[:])

        # ---- conv over row pairs
        with tc.tile_pool(name="convps", bufs=8, space="PSUM") as cps:
            staging = {}
            mm_cnt = {}
            mm_tot = {}

            def emit_block(us):
                tiles = {}
                for u in us:
                    tiles[u] = cps.tile([128, W], F32, name="pair_ps")
                    mm_cnt[u] = 0
                    mm_tot[u] = sum(
                        1 for wh, dx in _WSEQ
                        if (wh == 0 and u >= 0) or (wh == 1 and u <= 126))
                for wh, dx in _WSEQ:
                    for u in us:
                        if wh == 0 and u < 0:
                            continue
                        if wh == 1 and u > 126:
                            continue
                        rhs = slot_rhs(u + (0 if wh == 0 else 1), dx)
                        nc.tensor.matmul(
                            tiles[u][:], t_sb[2 * dx + wh][:], rhs,
                            start=(mm_cnt[u] == 0),
                            stop=(mm_cnt[u] == mm_tot[u] - 1),
                            skip_group_check=True)
                        mm_cnt[u] += 1
                for u in us:
                    gi, si = (u + 1) // GRP, (u + 1) % GRP
                    if gi not in staging:
                        staging[gi] = spool.tile([128, GRP * W], F32, name="staging")
                    st = staging[gi][:, si * W:(si + 1) * W]
                    if u == -1:
                        nc.scalar.copy(st[64:128, :], tiles[u][64:128, :])
                    elif u == 127:
                        nc.scalar.copy(st[0:64, :], tiles[u][0:64, :])
                    elif u % 2 == 0:
                        nc.vector.tensor_copy(st, tiles[u][:])
                    else:
                        nc.scalar.copy(st, tiles[u][:])
                    if si == GRP - 1 or u == 127:
                        emit_stores(gi)

            def emit_stores(gi):
                stg = staging.pop(gi)
                ua = max(0, gi * GRP - 1)           # first full pair in group
                ub = min(126, gi * GRP + GRP - 2)   # last full pair in group
                sv = stg.rearrange("p (g w) -> p g w", w=W)
                if gi == 0:
                    # row 0 from pair u=-1 (slot 0, upper half)
                    nc.gpsimd.dma_start(out=out[:, 0:1, :], in_=sv[64:128, 0:1, :])
                if ub >= ua:
                    G = ub - ua + 1
                    sa = ua + 1 - gi * GRP
                    nc.gpsimd.dma_start(
                        out=out[:, 2 * ua + 1: 2 * ub + 2: 2, :],
                        in_=sv[0:64, sa:sa + G, :])
                    nc.gpsimd.dma_start(
                        out=out[:, 2 * ua + 2: 2 * ub + 3: 2, :],
                        in_=sv[64:128, sa:sa + G, :])
                if gi == 128 // GRP:
                    # row 255 from pair u=127 (lower half)
                    si = 128 % GRP
                    nc.gpsimd.dma_start(out=out[:, H - 1:H, :],
                                        in_=sv[0:64, si:si + 1, :])

            emit_block([-1])
            for b0 in range(0, 128, BLK):
                emit_block(list(range(b0, b0 + BLK)))


_NC_CACHE = None


def _build_nc():
    global _NC_CACHE
    if _NC_CACHE is None:
        nc = bacc.Bacc("TRN2", target_bir_lowering=False, debug=False,
                       num_devices=B)
        with tile.TileContext(nc) as tc:
            _emit_kernel(tc)
        nc.compile()
        _NC_CACHE = nc
    return _NC_CACHE


def _in_maps(x, k_att_logits, dft_weight):
    x = np.asarray(x, dtype=np.float32)
    xbf = np.ascontiguousarray(x.astype(ml_dtypes.bfloat16))
    lg = np.asarray(k_att_logits, dtype=np.float32)
    dw = np.asarray(dft_weight, dtype=np.float32)
    # host-side gather: dense half-spectrum layout [k, h, w, c]
    dftP = dw[:, _INV, :].reshape(KNUM, D1, D2R, 2)
    dre = np.ascontiguousarray(dftP[..., 0].reshape(KNUM * D1, D2R))
    dim = np.ascontiguousarray(dftP[..., 1].reshape(KNUM * D1, D2R))
    maps = []
    for b in range(B):
        maps.append({
            "x_in": xbf[b],
            "lg_in": lg[b:b + 1],
            "dre_in": dre,
            "dim_in": dim,
            "cb_in": _C2,
            "sb_in": _S2,
            "ab_in": _AB,
        })
    return maps


def _execute(x, k_att_logits, dft_weight, trace=False, **trace_kwargs):
    nc = _build_nc()
    res = run_bass_kernel_spmd(
        nc, _in_maps(x, k_att_logits, dft_weight),
        core_ids=list(range(B)), trace=trace, **trace_kwargs)
    out = np.stack([r["out"] for r in res.results], axis=0)
    return out, res


def kernel(x, k_att_logits, dft_weight):
    out, _ = _execute(x, k_att_logits, dft_weight)
    return out.astype(np.float32)


# revision 7
# speedup vs baseline: 1.2297x; 1.1020x over previous
"""Trainium2 Bass kernel for nn_FDConv (per-sample frequency-domain-synthesized
3x3 grouped conv).

Strategy (data-parallel over batch, 1 sample per NeuronCore):
  - host: permute dft_weight into dense half-spectrum layout (pure gather),
    precompute DFT basis matrices as constants, stage x as bf16 in a padded
    parity-split layout so every load descriptor is one 8KB contiguous run.
  - device per core:
      att = sigmoid(logits)                      (0.5 att scale folded in basis)
      GT  = (att-mixed spectrum)^T . [C | S]     (stage-1 iFFT along axis 0 via
                                                  PE matmuls; kernel-mixing att
                                                  contraction folded in)
      T[which,dx] = A_dx^T . GT_re - B_dx^T . GT_im  (stage-2 irfft + reshape to
                                                  six 128x128 conv weight mats)
      conv: x bf16 with even rows on partitions 0-63 and odd rows on 64-127;
            output row pairs (2u+1, 2u+2) computed as six K=128/M=128/N=256
            matmuls (T1_dx on slot u, T2_dx on slot u+1) accumulated in PSUM.
  - outputs copied PSUM->SBUF (DVE/ACT) and DMA'd back fp32 via SWDGE.
"""

import numpy as np
import ml_dtypes

import concourse.bass as bass
import concourse.bacc as bacc
import concourse.tile as tile
import concourse.mybir as mybir
from concourse.bass_utils import run_bass_kernel_spmd

F32 = mybir.dt.float32
BF16 = mybir.dt.bfloat16

B, CIN, COUT, KS = 8, 64, 64, 3
H, W = 256, 256
KNUM = 4
D1, D2 = COUT * KS, CIN * KS          # 192, 192
D2R = D2 // 2 + 1                     # 97
NF = D1 * D2R                         # 18624

NPAIR = 128          # output row pairs (2u+1, 2u+2), u = -1..127
SLOT = W + 2         # 258: [pad, 256 cols, pad] per row-slot
CHS = 16             # slots per x chunk
NCH = NPAIR // CHS   # 8 chunks
GRP = 16             # pairs per store group
BLK = 4              # pairs per PE block

# constant-pack column offsets (fp32, 97 partitions)
_O_DRE = 0
_O_DIM = 8 * D2R              # 776
_O_CB = 2 * 8 * D2R           # 1552
_O_SB = _O_CB + 2 * D1        # 1936
_PACKW = _O_SB + 2 * D1       # 2320


def _host_constants():
    fh = np.fft.fftfreq(D1)
    fw = np.fft.rfftfreq(D2)
    dist = np.sqrt(fh[:, None] ** 2 + fw[None, :] ** 2)
    idx = np.argsort(dist.ravel(), kind='stable')
    FH = (idx // D2R).astype(np.int64)
    FW = (idx % D2R).astype(np.int64)
    perm = FH * D2R + FW
    inv = np.empty(NF, dtype=np.int64)
    inv[perm] = np.arange(NF)

    hh = np.arange(D1)
    ang = 2.0 * np.pi * np.outer(hh, hh) / D1
    # att scale 2/KNUM = 0.5 folded into the stage-1 basis
    Cb = (np.cos(ang) * (0.5 / D1)).astype(np.float32)
    Sb = (np.sin(ang) * (0.5 / D1)).astype(np.float32)
    C2 = np.concatenate([Cb[:96], Cb[96:]], axis=1)           # [96, 384]
    S2 = np.concatenate([Sb[:96], Sb[96:]], axis=1)           # [96, 384]

    w_ = np.arange(D2R)
    n_ = np.arange(D2)
    alpha = np.full(D2R, 2.0); alpha[0] = 1.0; alpha[-1] = 1.0
    beta = np.full(D2R, 2.0); beta[0] = 0.0; beta[-1] = 0.0
    ang2 = 2.0 * np.pi * np.outer(w_, n_) / D2
    A = (alpha[:, None] * np.cos(ang2) / D2).astype(np.float32)   # [97, 192]
    Bm = (beta[:, None] * np.sin(ang2) / D2).astype(np.float32)
    ab = np.concatenate(
        [A[:, dx::3] for dx in range(3)] + [-Bm[:, dx::3] for dx in range(3)],
        axis=1,
    ).astype(ml_dtypes.bfloat16)                               # [97, 384]
    return inv, C2, S2, ab


_INV, _C2, _S2, _AB = _host_constants()

# (which, dx) order used in the conv weight loop; t_sb index = 2*dx + which
_WSEQ = [(0, 0), (1, 0), (0, 1), (1, 1), (0, 2), (1, 2)]
# valid quadrants (J, r, c0) per `which` (c0 = J - r + 2*which)
_QUADS = {
    0: [(0, 0, 0), (1, 0, 1), (1, 1, 0)],
    1: [(0, 0, 2), (0, 1, 1), (1, 1, 2)],
}
_ZQUAD = {0: (0, 1), 1: (1, 0)}  # zero quadrant (J, r)


def _emit_kernel(tc):
    nc = tc.nc
    from contextlib import ExitStack

    # x: [parity, cin, slot, 258] bf16, host-padded (col 0 and 257 are zeros)
    x_in = nc.dram_tensor("x_in", [2, CIN, NPAIR, SLOT], BF16,
                          kind="ExternalInput").ap()
    lg_in = nc.dram_tensor("lg_in", [1, KNUM], F32, kind="ExternalInput").ap()
    pk_in = nc.dram_tensor("pk_in", [D2R, _PACKW], F32, kind="ExternalInput").ap()
    ab_in = nc.dram_tensor("ab_in", [D2R, 6 * 64], BF16, kind="ExternalInput").ap()
    out = nc.dram_tensor("out", [COUT, H, W], F32, kind="ExternalOutput").ap()

    with ExitStack() as ctx:
        cpool = ctx.enter_context(tc.tile_pool(name="cpool", bufs=1))
        xbpool = ctx.enter_context(tc.tile_pool(name="xbpool", bufs=4))
        spool = ctx.enter_context(tc.tile_pool(name="spool", bufs=3))

        # ---- small input loads (HWDGE sync queue)
        l_sb = cpool.tile([1, KNUM], F32, name="l_sb")
        nc.sync.dma_start(out=l_sb[:], in_=lg_in)
        pk_sb = cpool.tile([D2R, _PACKW], F32, name="pk_sb")
        nc.sync.dma_start(out=pk_sb[:], in_=pk_in)
        ab_sb = cpool.tile([D2R, 6 * 64], BF16, name="ab_sb")
        nc.sync.dma_start(out=ab_sb[:], in_=ab_in)
        dre32 = pk_sb[0:96, _O_DRE:_O_DRE + 8 * D2R]
        dim32 = pk_sb[0:96, _O_DIM:_O_DIM + 8 * D2R]
        cb_sb = pk_sb[0:96, _O_CB:_O_CB + 2 * D1]
        sb_sb = pk_sb[0:96, _O_SB:_O_SB + 2 * D1]

        # ---- att = sigmoid(logits); broadcast via K=1 matmul with ones
        sig_sb = cpool.tile([1, KNUM], F32, name="sig_sb")
        nc.scalar.activation(sig_sb[:], l_sb[:], mybir.ActivationFunctionType.Sigmoid)
        ones_sb = cpool.tile([1, 128], F32, name="ones_sb")
        nc.vector.memset(ones_sb[:], 1.0)

        # ---- x chunk loads (bf16 HWDGE): slot t holds rows (2t, 2t+1)
        xch = []
        for c in range(NCH):
            xb = xbpool.tile([128, CHS * SLOT], BF16, name="xb")
            for par in range(2):
                nc.sync.dma_start(
                    out=xb[64 * par: 64 * par + 64, :],
                    in_=x_in[par, :, c * CHS:(c + 1) * CHS, :])
            xch.append(xb)

        def slot_rhs(s, dx):
            c, loc = s // CHS, s % CHS
            return xch[c][:, loc * SLOT + dx: loc * SLOT + dx + W]

        # ---- bf16 casts of the permuted spectrum
        dre_sb = cpool.tile([96, 8 * D2R], BF16, name="dre_sb")
        nc.vector.tensor_copy(dre_sb[:], dre32)
        dim_sb = cpool.tile([96, 8 * D2R], BF16, name="dim_sb")
        nc.vector.tensor_copy(dim_sb[:], dim32)
        dimneg_sb = cpool.tile([96, 8 * D2R], BF16, name="dimneg_sb")
        nc.vector.tensor_scalar_mul(dimneg_sb[:], dim32, -1.0)

        gtre_sb = cpool.tile([D2R, D1 + 2], BF16, name="gtre_sb")
        gtim_sb = cpool.tile([D2R, D1 + 2], BF16, name="gtim_sb")
        t_sb = [cpool.tile([128, 128], BF16, name=f"t_sb_{i}") for i in range(6)]

        with tc.tile_pool(name="dftps", bufs=1, space="PSUM") as dpool:
            # att broadcast: [128, 4] = ones.T @ sig
            att_ps = dpool.tile([128, KNUM], F32, name="att_ps")
            nc.tensor.matmul(att_ps[:], ones_sb[:], sig_sb[:], start=True, stop=True)
            att_sb = cpool.tile([96, KNUM], F32, name="att_sb")
            nc.vector.tensor_copy(att_sb[:], att_ps[0:96, :])

            # attC on ACT, attS on DVE (chunk ck = 2k + half)
            attC = cpool.tile([96, 8 * D1], BF16, name="attC")
            attS = cpool.tile([96, 8 * D1], BF16, name="attS")
            for ck in range(8):
                k, half = ck // 2, ck % 2
                nc.scalar.mul(
                    attC[:, ck * D1:(ck + 1) * D1],
                    cb_sb[:, half * D1:(half + 1) * D1],
                    att_sb[:, k:k + 1])
                nc.vector.tensor_scalar_mul(
                    attS[:, ck * D1:(ck + 1) * D1],
                    sb_sb[:, half * D1:(half + 1) * D1],
                    att_sb[:, k:k + 1])

            # ---- stage 1: GT = F^T . (att*C) etc, contraction over (k, h)
            gtre_ps = dpool.tile([D2R, D1], F32, name="gtre_ps")
            gtim_ps = dpool.tile([D2R, D1], F32, name="gtim_ps")
            for ck in range(8):
                nc.tensor.matmul(
                    gtre_ps[:], dre_sb[:, ck * D2R:(ck + 1) * D2R],
                    attC[:, ck * D1:(ck + 1) * D1],
                    start=(ck == 0), stop=False)
            for ck in range(8):
                nc.tensor.matmul(
                    gtre_ps[:], dimneg_sb[:, ck * D2R:(ck + 1) * D2R],
                    attS[:, ck * D1:(ck + 1) * D1],
                    start=False, stop=(ck == 7))
            for ck in range(8):
                nc.tensor.matmul(
                    gtim_ps[:], dre_sb[:, ck * D2R:(ck + 1) * D2R],
                    attS[:, ck * D1:(ck + 1) * D1],
                    start=(ck == 0), stop=False)
            for ck in range(8):
                nc.tensor.matmul(
                    gtim_ps[:], dim_sb[:, ck * D2R:(ck + 1) * D2R],
                    attC[:, ck * D1:(ck + 1) * D1],
                    start=False, stop=(ck == 7))
            nc.vector.tensor_copy(gtre_sb[:, 0:D1], gtre_ps[:])
            nc.vector.tensor_copy(gtim_sb[:, 0:D1], gtim_ps[:])

            # ---- stage 2: six conv weight matrices T[(ci,j),(co,r)]
            def gview(g, c0):
                return g[:, c0:c0 + D1].rearrange(
                    "w (co th) -> w co th", th=3)[:, :, 0:1]

            for i, (which, dx) in enumerate(_WSEQ):
                t_ps = dpool.tile([128, 128], F32, name="t_ps", bufs=3)
                zj, zr = _ZQUAD[which]
                nc.vector.memset(t_ps[64 * zj:64 * zj + 64, 64 * zr:64 * zr + 64], 0.0)
                for (J, r, c0) in _QUADS[which]:
                    o = t_ps[64 * J:64 * J + 64, 64 * r:64 * r + 64]
                    nc.tensor.matmul(o, ab_sb[:, dx * 64:(dx + 1) * 64],
                                     gview(gtre_sb, c0), start=True, stop=False)
                    nc.tensor.matmul(o, ab_sb[:, (3 + dx) * 64:(4 + dx) * 64],
                                     gview(gtim_sb, c0), start=False, stop=True)
                nc.vector.tensor_copy(t_sb[2 * dx + which][:], t_ps[:])

        # ---- conv over row pairs
        with tc.tile_pool(name="convps", bufs=8, space="PSUM") as cps:
            staging = {}
            mm_cnt = {}
            mm_tot = {}

            def emit_block(us):
                tiles = {}
                for u in us:
                    tiles[u] = cps.tile([128, W], F32, name="pair_ps")
                    mm_cnt[u] = 0
                    mm_tot[u] = sum(
                        1 for wh, dx in _WSEQ
                        if (wh == 0 and u >= 0) or (wh == 1 and u <= 126))
                for wh, dx in _WSEQ:
                    for u in us:
                        if wh == 0 and u < 0:
                            continue
                        if wh == 1 and u > 126:
                            continue
                        rhs = slot_rhs(u + (0 if wh == 0 else 1), dx)
                        nc.tensor.matmul(
                            tiles[u][:], t_sb[2 * dx + wh][:], rhs,
                            start=(mm_cnt[u] == 0),
                            stop=(mm_cnt[u] == mm_tot[u] - 1),
                            skip_group_check=True)
                        mm_cnt[u] += 1
                for u in us:
                    gi, si = (u + 1) // GRP, (u + 1) % GRP
                    if gi not in staging:
                        staging[gi] = spool.tile([128, GRP * W], F32, name="staging")
                    st = staging[gi][:, si * W:(si + 1) * W]
                    if u == -1:
                        nc.scalar.copy(st[64:128, :], tiles[u][64:128, :])
                    elif u == 127:
                        nc.scalar.copy(st[0:64, :], tiles[u][0:64, :])
                    elif u % 2 == 0:
                        nc.vector.tensor_copy(st, tiles[u][:])
                    else:
                        nc.scalar.copy(st, tiles[u][:])
                    if si == GRP - 1 or u == 127:
                        emit_stores(gi)

            def emit_stores(gi):
                stg = staging.pop(gi)
                ua = max(0, gi * GRP - 1)           # first full pair in group
                ub = min(126, gi * GRP + GRP - 2)   # last full pair in group
                sv = stg.rearrange("p (g w) -> p g w", w=W)
                if gi == 0:
                    # row 0 from pair u=-1 (slot 0, upper half)
                    nc.gpsimd.dma_start(out=out[:, 0:1, :], in_=sv[64:128, 0:1, :])
                if ub >= ua:
                    G = ub - ua + 1
                    sa = ua + 1 - gi * GRP
                    nc.gpsimd.dma_start(
                        out=out[:, 2 * ua + 1: 2 * ub + 2: 2, :],
                        in_=sv[0:64, sa:sa + G, :])
                    nc.gpsimd.dma_start(
                        out=out[:, 2 * ua + 2: 2 * ub + 3: 2, :],
                        in_=sv[64:128, sa:sa + G, :])
                if gi == 128 // GRP:
                    # row 255 from pair u=127 (lower half)
                    si = 128 % GRP
                    nc.gpsimd.dma_start(out=out[:, H - 1:H, :],
                                        in_=sv[0:64, si:si + 1, :])

            emit_block([-1])
            for b0 in range(0, 128, BLK):
                emit_block(list(range(b0, b0 + BLK)))


_NC_CACHE = None


def _build_nc():
    global _NC_CACHE
    if _NC_CACHE is None:
        nc = bacc.Bacc("TRN2", target_bir_lowering=False, debug=False,
                       num_devices=B)
        with tile.TileContext(nc) as tc:
            _emit_kernel(tc)
        nc.compile()
        _NC_CACHE = nc
    return _NC_CACHE


def _in_maps(x, k_att_logits, dft_weight):
    x = np.asarray(x, dtype=np.float32)
    lg = np.asarray(k_att_logits, dtype=np.float32)
    dw = np.asarray(dft_weight, dtype=np.float32)

    # x -> bf16, parity-split rows, host-inserted zero pad columns
    xp = np.zeros((B, 2, CIN, NPAIR, SLOT), dtype=ml_dtypes.bfloat16)
    xv = x.reshape(B, CIN, NPAIR, 2, W).transpose(0, 3, 1, 2, 4)  # [b,j,c,t,w]
    xp[:, :, :, :, 1:1 + W] = xv.astype(ml_dtypes.bfloat16)

    # host-side gather: dense half-spectrum layout [k, h, w, c], chunk-major
    dftP = dw[:, _INV, :].reshape(KNUM, 2, 96, D2R, 2)   # [k, half, p, w, c]
    dre = dftP[..., 0].transpose(2, 0, 1, 3).reshape(96, 8 * D2R)
    dim = dftP[..., 1].transpose(2, 0, 1, 3).reshape(96, 8 * D2R)
    pk = np.zeros((D2R, _PACKW), dtype=np.float32)
    pk[0:96, _O_DRE:_O_DRE + 8 * D2R] = dre
    pk[0:96, _O_DIM:_O_DIM + 8 * D2R] = dim
    pk[0:96, _O_CB:_O_CB + 2 * D1] = _C2
    pk[0:96, _O_SB:_O_SB + 2 * D1] = _S2

    maps = []
    for b in range(B):
        maps.append({
            "x_in": np.ascontiguousarray(xp[b]),
            "lg_in": lg[b:b + 1],
            "pk_in": pk,
            "ab_in": _AB,
        })
    return maps


def _execute(x, k_att_logits, dft_weight, trace=False, **trace_kwargs):
    nc = _build_nc()
    res = run_bass_kernel_spmd(
        nc, _in_maps(x, k_att_logits, dft_weight),
        core_ids=list(range(B)), trace=trace, **trace_kwargs)
    out = np.stack([r["out"] for r in res.results], axis=0)
    return out, res


def kernel(x, k_att_logits, dft_weight):
    out, _ = _execute(x, k_att_logits, dft_weight)
    return out.astype(np.float32)
